# revision 1
# baseline (speedup 1.0000x reference)
"""Trainium2 Bass kernel for nn_DoubleConv (modulated deformable conv v2 x2 + BN + ReLU).

Sharding: data-parallel over (sample n, image half) -> 8 shards on 8 NeuronCores.
Each core computes both layers for its 48-row slice (with recomputed halo rows for
layer-2 sampling); training-mode BatchNorm statistics are made exact with a tiny
cross-core AllReduce of per-channel (sum, sumsq).

Self-contained: hardcodes all shapes from the problem spec.
"""

import numpy as np

import concourse.bass as bass
import concourse.bacc as bacc
import concourse.mybir as mybir
import concourse.tile as tile
from concourse import bass_utils

F32 = mybir.dt.float32
F32R = mybir.dt.float32r
BF16 = mybir.dt.bfloat16
I16 = mybir.dt.int16
ALU = mybir.AluOpType
ACTF = mybir.ActivationFunctionType

# ---------------- geometry ----------------
N, CIN, CMID, COUT, H, W = 4, 64, 128, 128, 96, 96
K = 9
NCORES = 8
OWN = 48                      # own image rows per core
MR, MC = 8, 4                 # plane row/col margins
WP = W + 2 * MC               # 104 padded width
PH = OWN + 2 * MR             # 64 plane rows
PLANE = PH * WP               # 6656
L1R0, L1NR = 4, 56            # layer-1 computed plane rows [4, 60)
L2R0, L2NR = 8, 48            # layer-2 (own) plane rows [8, 56)
L1PX = L1NR * W               # 5376
L2PX = L2NR * W               # 4608
CH = 384                      # pixel chunk (4 rows x 96)
L1NC, L2NC = L1PX // CH, L2PX // CH   # 14, 12 chunks
GRP = 768                     # gather group (2 chunks)
L1NG, L2NG = L1PX // GRP, L2PX // GRP  # 7, 6 groups
NE = PLANE - WP - 2           # ap_gather num_elems (max corner shift WP+1)
CNT = float(N * H * W)        # BN count 36864
EPS = 1e-5

SHIFTS = [0, 1, WP, WP + 1]   # corner ab -> flat index shift (a*WP + b)


def _plane_pad(img, r0):
    """img [C, 96, 96] -> padded plane [C, PH, WP] for own rows [r0, r0+48)."""
    C = img.shape[0]
    out = np.zeros((C, PH, WP), np.float32)
    lo, hi = r0 - MR, r0 + OWN + MR
    slo, shi = max(lo, 0), min(hi, H)
    out[:, slo - lo:shi - lo, MC:MC + W] = img[:, slo:shi, :]
    return out


def _host_prep(inputs):
    """Build the 8 per-core input maps (all numpy)."""
    x = np.asarray(inputs['x'], np.float32)
    w1 = np.asarray(inputs['w1'], np.float32)
    off_w1 = np.asarray(inputs['off_w1'], np.float32)
    off_b1 = np.asarray(inputs['off_b1'], np.float32)
    g1 = np.asarray(inputs['gamma1'], np.float32)
    b1 = np.asarray(inputs['beta1'], np.float32)
    w2 = np.asarray(inputs['w2'], np.float32)
    off_w2 = np.asarray(inputs['off_w2'], np.float32)
    off_b2 = np.asarray(inputs['off_b2'], np.float32)
    g2 = np.asarray(inputs['gamma2'], np.float32)
    b2 = np.asarray(inputs['beta2'], np.float32)

    ky = np.arange(K) // 3 - 1
    kx = np.arange(K) % 3 - 1

    import ml_dtypes as _mld
    # offset conv weights, output channels permuted to (py x9, px x9, mlogit x9)
    perm = list(range(0, 18, 2)) + list(range(1, 18, 2)) + list(range(18, 27))

    def off_lhsT(ow, cin):
        owp = ow[perm]                       # [27, cin, 3, 3]
        t = np.zeros((K, cin, 27), np.float32)
        for t_i in range(K):
            ty, tx = t_i // 3 - 1, t_i % 3 - 1
            t[t_i] = owp[:, :, ty + 1, tx + 1].T
        return t.astype(_mld.bfloat16)        # [9, cin, 27]

    offw1_t = off_lhsT(off_w1, CIN)
    offw2_t = off_lhsT(off_w2, CMID)

    # main conv lhsT blocks
    w1k = w1.reshape(CMID, CIN, K)
    w2k = w2.reshape(COUT, CMID, K)
    w1p = np.zeros((5, 128, 128), np.float32)
    for b in range(4):
        w1p[b, :64] = w1k[:, :, 2 * b].T
        w1p[b, 64:] = w1k[:, :, 2 * b + 1].T
    w1p[4, :64] = w1k[:, :, 8].T
    w1p = w1p.astype(_mld.bfloat16)
    w2p = np.stack([w2k[:, :, k].T for k in range(K)]).astype(_mld.bfloat16)

    import ml_dtypes
    # one-hot selectors for V replication: Vrep[m, px] = V36[sel(m), px]
    # L1 blocks 0-3: m -> tap 2b + m//64 ; block 4: m<64 -> tap 8 ; L2 block k: m -> k
    vsel1 = np.zeros((5, 4, 36, 128), np.float32)
    for b in range(4):
        for ab in range(4):
            vsel1[b, ab, ab * 9 + 2 * b, :64] = 1.0
            vsel1[b, ab, ab * 9 + 2 * b + 1, 64:] = 1.0
    for ab in range(4):
        vsel1[4, ab, ab * 9 + 8, :64] = 1.0
    vsel2 = np.zeros((9, 4, 36, 128), np.float32)
    for k in range(K):
        for ab in range(4):
            vsel2[k, ab, ab * 9 + k, :] = 1.0
    vsel1 = vsel1.reshape(20, 36, 128).astype(ml_dtypes.bfloat16)
    vsel2 = vsel2.reshape(36, 36, 128).astype(ml_dtypes.bfloat16)

    # stacked per-pixel constant maps, layout [(k, chunk), CH]
    def grids(r0, nrows, prow0, nch, offb):
        pr = prow0 + np.arange(nrows)              # plane rows
        pc = MC + np.arange(W)                     # plane cols
        gy = np.broadcast_to(pr[:, None], (nrows, W)).reshape(-1).astype(np.float32)
        gx = np.broadcast_to(pc[None, :], (nrows, W)).reshape(-1).astype(np.float32)
        gy_st = np.zeros((K * nch, CH), np.float32)
        gx_st = np.zeros((K * nch, CH), np.float32)
        for k in range(K):
            for c in range(nch):
                gy_st[k * nch + c] = gy[c * CH:(c + 1) * CH] + ky[k] + offb[2 * k]
                gx_st[k * nch + c] = gx[c * CH:(c + 1) * CH] + kx[k] + offb[2 * k + 1]
        return gy_st, gx_st

    in_maps = []
    for core in range(NCORES):
        n, half = core // 2, core % 2
        r0 = half * OWN
        gy1, gx1 = grids(r0, L1NR, L1R0, L1NC, off_b1)
        gy2, gx2 = grids(r0, L2NR, L2R0, L2NC, off_b2)
        mb1 = np.repeat(off_b1[18:27], L1NC).astype(np.float32)[:, None]
        mb2 = np.repeat(off_b2[18:27], L2NC).astype(np.float32)[:, None]

        topv = np.full((128, 1), 0.0 if r0 == 0 else 1.0, np.float32)
        botv = np.full((128, 1), 0.0 if r0 + OWN >= H else 1.0, np.float32)

        in_maps.append({
            'x_p': _plane_pad(x[n], r0).reshape(CIN, PLANE).astype(_mld.bfloat16),
            'x_pf': np.concatenate([_plane_pad(x[n], r0).reshape(CIN, PLANE)] * 2, 0),
            'gy1': gy1, 'gx1': gx1, 'mb1': mb1,
            'gy2': gy2, 'gx2': gx2, 'mb2': mb2,
            'offw1': offw1_t, 'offw2': offw2_t,
            'w1p': w1p, 'w2p': w2p,
            'vsel1': vsel1, 'vsel2': vsel2,
            'topv': topv, 'botv': botv,
            'g1': g1[:, None].copy(), 'b1': b1[:, None].copy(),
            'g2': g2[:, None].copy(), 'b2': b2[:, None].copy(),
        })
    return in_maps


# ---------------- module build ----------------

def _deform_layer(nc, pools, cfg):
    """Emit one modulated-deformable-conv layer + BN stats/apply."""
    cin = cfg['cin']
    nch, ngr = cfg['nchunks'], cfg['ngroups']
    nk_st = K * nch                    # stacked rows (126 / 108)
    px_all = nch * CH
    prow0 = cfg['prow0']
    wseg = ngr * 48
    sb, rot, psum, dram = pools['sb'], pools['rot'], pools['psum'], pools['dram']
    L = cfg['layer']

    # ---- offset conv: 9 accumulated matmuls per chunk -> off_sb (bf16) ----
    off_sb = sb.tile([32, px_all], BF16, tag=f'off_sb{L}')
    for c in range(nch):
        po = psum.tile([27, CH], F32, tag='psum_off')
        base = (prow0 + 4 * c) * WP + MC
        for t in range(K):
            ty, tx = t // 3 - 1, t % 3 - 1
            sh = ty * WP + tx
            rhs = cfg['src'][0:cin, base + sh: base + sh + 4 * WP].rearrange(
                'p (r w) -> p r w', w=WP)[:, :, 0:W]
            lhsT = cfg['offw'][0:cin, t * 27:(t + 1) * 27]
            nc.tensor.matmul(po[:, :], lhsT, rhs,
                             start=(t == 0), stop=(t == K - 1))
        nc.scalar.copy(off_sb[0:27, c * CH:(c + 1) * CH], po[:, :])

    # ---- stack (k,chunk) onto partitions via DRAM hop ----
    dB = dram.tile([27, px_all], BF16, tag=f'dB{L}')
    nc.sync.dma_start(dB[:, :], off_sb[0:27, :])
    dy_st = sb.tile([nk_st, CH], BF16, tag='dy_st')
    dx_st = sb.tile([nk_st, CH], BF16, tag='dx_st')
    ml_st = sb.tile([nk_st, CH], BF16, tag='ml_st')
    for (dst, p0) in ((dy_st, 0), (dx_st, 9), (ml_st, 18)):
        src = dB[p0:p0 + 9, :].rearrange('k (c u) -> (k c) u', c=nch)
        nc.sync.dma_start(dst[0:nk_st, :], src)

    # ---- per-pixel prep on stacked tiles ----
    py = sb.tile([nk_st, CH], F32, tag='py')
    px = sb.tile([nk_st, CH], F32, tag='px')
    ly = sb.tile([nk_st, CH], F32, tag='ly')
    lx = sb.tile([nk_st, CH], F32, tag='lx')
    m_st = sb.tile([nk_st, CH], F32, tag='m_st')
    idxf = sb.tile([nk_st, CH], F32, tag='idxf')
    idxi = sb.tile([nk_st, CH], I16, tag='idxi')
    tmp = sb.tile([nk_st, CH], F32, tag='tmp')
    wx0 = sb.tile([nk_st, CH], F32, tag='wx0')
    V = sb.tile([nk_st, 4 * CH], BF16, tag='V')

    A = lambda t: t[0:nk_st, :]
    nc.vector.tensor_tensor(A(py), A(dy_st), cfg['gy'][0:nk_st, :], ALU.add)
    nc.vector.tensor_tensor(A(px), A(dx_st), cfg['gx'][0:nk_st, :], ALU.add)
    # floor via round-to-nearest magic + compare (py, px always > 0 here)
    MAGIC = 12582912.0
    y0 = sb.tile([nk_st, CH], F32, tag='y0')
    x0 = sb.tile([nk_st, CH], F32, tag='x0')
    nc.vector.tensor_scalar(A(y0), A(py), MAGIC, None, ALU.add)
    nc.vector.tensor_scalar(A(y0), A(y0), -MAGIC, None, ALU.add)
    nc.vector.tensor_tensor(A(tmp), A(y0), A(py), ALU.is_gt)
    nc.vector.tensor_tensor(A(y0), A(y0), A(tmp), ALU.subtract)
    nc.vector.tensor_scalar(A(x0), A(px), MAGIC, None, ALU.add)
    nc.vector.tensor_scalar(A(x0), A(x0), -MAGIC, None, ALU.add)
    nc.vector.tensor_tensor(A(tmp), A(x0), A(px), ALU.is_gt)
    nc.vector.tensor_tensor(A(x0), A(x0), A(tmp), ALU.subtract)
    nc.vector.tensor_tensor(A(ly), A(py), A(y0), ALU.subtract)
    nc.vector.tensor_tensor(A(lx), A(px), A(x0), ALU.subtract)
    nc.scalar.activation(A(m_st), A(ml_st), ACTF.Sigmoid, bias=cfg['mb'][0:nk_st, :])
    # idx00 = y0*WP + x0, clamped to [0, NE-1]
    nc.vector.tensor_scalar(A(idxf), A(y0), float(WP), None, ALU.mult)
    nc.vector.tensor_tensor(A(idxf), A(idxf), A(x0), ALU.add)
    nc.vector.tensor_scalar(A(idxf), A(idxf), 0.0, float(NE - 1), ALU.max, ALU.min)
    nc.vector.tensor_copy(idxi[0:nk_st, :], A(idxf))

    # V[:, ab*CH:(ab+1)*CH] = m * wy_a * wx_b
    nc.vector.tensor_scalar(A(tmp), A(ly), 1.0, -1.0, ALU.subtract, ALU.mult)
    nc.vector.tensor_tensor(A(tmp), A(tmp), A(m_st), ALU.mult)    # m*(1-ly)
    nc.vector.tensor_tensor(A(idxf), A(ly), A(m_st), ALU.mult)    # m*ly (reuse idxf)
    nc.vector.tensor_scalar(A(wx0), A(lx), 1.0, -1.0, ALU.subtract, ALU.mult)
    nc.vector.tensor_tensor(V[0:nk_st, 0 * CH:1 * CH], A(tmp), A(wx0), ALU.mult)
    nc.vector.tensor_tensor(V[0:nk_st, 1 * CH:2 * CH], A(tmp), A(lx), ALU.mult)
    nc.vector.tensor_tensor(V[0:nk_st, 2 * CH:3 * CH], A(idxf), A(wx0), ALU.mult)
    nc.vector.tensor_tensor(V[0:nk_st, 3 * CH:4 * CH], A(idxf), A(lx), ALU.mult)

    if cfg.get('dbg'):
        d = cfg['dbg']
        nc.sync.dma_start(d['off'].ap(), off_sb[0:27, :])
        nc.sync.dma_start(d['dyst'].ap(), dy_st[0:nk_st, :])
        nc.sync.dma_start(d['dxst'].ap(), dx_st[0:nk_st, :])
        nc.sync.dma_start(d['py'].ap(), py[0:nk_st, :])
        nc.sync.dma_start(d['px'].ap(), px[0:nk_st, :])
        nc.sync.dma_start(d['m'].ap(), m_st[0:nk_st, :])
        nc.sync.dma_start(d['idx'].ap(), idxi[0:nk_st, :])

    # ---- V36 [(ab,k), px_all] via DRAM reshape hop ----
    dV = dram.tile([nk_st, 4 * CH], BF16, tag=f'dV{L}')
    nc.sync.dma_start(dV[:, :], V[0:nk_st, :])
    v36 = sb.tile([36, px_all], BF16, tag='v36')
    for ab in range(4):
        src = dV[:, ab * CH:(ab + 1) * CH].rearrange('(k c) u -> k c u', c=nch)
        nc.sync.dma_start(v36[ab * 9:ab * 9 + 9, :], src)

    if cfg.get('dbg'):
        nc.sync.dma_start(cfg['dbg']['v36'].ap(), v36[0:36, :])

    # ---- wrapped int16 indices via DRAM hop ----
    dA = dram.tile([nk_st, CH], I16, tag=f'dA{L}')
    nc.sync.dma_start(dA[:, :], idxi[0:nk_st, :])
    wrapped = sb.tile([128, K * wseg], I16, tag='wrapped')
    src = dA[:, :].rearrange('(k c) (u1 p) -> p k c u1', k=K, u1=24)
    dst = wrapped[0:16, 0:K * wseg].rearrange('p (k c u1) -> p k c u1', c=nch, u1=24)
    nc.sync.dma_start(dst, src)
    for g8 in range(1, 8):
        nc.sync.dma_start(wrapped[16 * g8:16 * g8 + 16, 0:K * wseg],
                          wrapped[0:16, 0:K * wseg])
    blocks = cfg['wblocks']
    if cin == 64:
        wblk = sb.tile([128, len(blocks) * wseg], I16, tag='wblk')
        for b, (_, taps, rows) in enumerate(blocks):
            t_lo, t_hi = taps[0], taps[-1]
            nc.sync.dma_start(wblk[0:64, b * wseg:(b + 1) * wseg],
                              wrapped[0:64, t_lo * wseg:(t_lo + 1) * wseg])
            nc.sync.dma_start(wblk[64:128, b * wseg:(b + 1) * wseg],
                              wrapped[0:64, t_hi * wseg:(t_hi + 1) * wseg])

    if cfg.get('dbg'):
        nc.sync.dma_start(cfg['dbg']['wr'].ap(), wrapped[:, 0:K * wseg])

    # ---- per group: gathers + Vrep + weighted sum -> S blocks; matmuls ----
    nblk = len(blocks)
    out_chunks = []
    for g in range(ngr):
        s_tiles = []
        for b, (_, taps, rows) in enumerate(blocks):
            S = sb.tile([128, GRP], BF16, tag=f'S{b}')
            for ab in range(4):
                G = rot.tile([128, GRP], F32, tag='Gbuf')
                if cin == 64:
                    nc.gpsimd.ap_gather(
                        G[0:128, :],
                        cfg['gsrc'][0:128, SHIFTS[ab]:SHIFTS[ab] + NE],
                        wblk[0:128, b * wseg + g * 48:b * wseg + (g + 1) * 48],
                        channels=128, num_elems=NE, d=1, num_idxs=GRP)
                else:
                    k = taps[0]
                    nc.gpsimd.ap_gather(
                        G[0:128, :],
                        cfg['gsrc'][0:128, SHIFTS[ab]:SHIFTS[ab] + NE],
                        wrapped[0:128, k * wseg + g * 48:k * wseg + (g + 1) * 48],
                        channels=128, num_elems=NE, d=1, num_idxs=GRP)
                vsel = cfg['vsel'][0:36, (b * 4 + ab) * 128:(b * 4 + ab + 1) * 128]
                for h in range(2):
                    pv = psum.tile([128, CH], F32, tag='psum_vrep')
                    nc.tensor.matmul(
                        pv[:, :], vsel,
                        v36[:, g * GRP + h * CH: g * GRP + (h + 1) * CH],
                        start=True, stop=True)
                    hs = slice(h * CH, (h + 1) * CH)
                    if cfg.get('dbg') and g == 0 and b == 0 and ab == 0:
                        pvc = sb.tile([128, CH], F32, tag='pvdbg')
                        nc.scalar.copy(pvc[:, :], pv[:, :])
                        nc.sync.dma_start(cfg['dbg']['pv0'].ap()[:, hs], pvc[:, :])
                    if ab == 0:
                        nc.vector.tensor_tensor(S[0:rows, hs], G[0:rows, hs],
                                                pv[0:rows, :], ALU.mult)
                    else:
                        T2 = rot.tile([128, CH], BF16, tag='Tbuf')
                        nc.vector.tensor_tensor(T2[0:rows, :], G[0:rows, hs],
                                                pv[0:rows, :], ALU.mult)
                        nc.vector.tensor_tensor(S[0:rows, hs], S[0:rows, hs],
                                                T2[0:rows, :], ALU.add)
            if cfg.get('dbg') and g == 0 and b == 0:
                nc.sync.dma_start(cfg['dbg']['G0'].ap(), G[:, :])
                nc.sync.dma_start(cfg['dbg']['S0'].ap(), S[:, :])
            s_tiles.append((S, rows))

        for h in range(2):
            c = g * 2 + h
            pm = psum.tile([128, CH], F32, tag='psum_main')
            for b, (wl, taps, rows) in enumerate(blocks):
                nc.tensor.matmul(pm[:, :], wl[0:rows, :],
                                 s_tiles[b][0][0:rows, h * CH:(h + 1) * CH],
                                 start=(b == 0), stop=(b == nblk - 1))
            # write pre-BN output
            if cfg['dst_plane'] is not None:
                base = (prow0 + 4 * c) * WP + MC
                dst = cfg['dst_plane'][:, base:base + 4 * WP].rearrange(
                    'p (r w) -> p r w', w=WP)[:, :, 0:W]
                nc.scalar.copy(dst, pm[:, :].rearrange('p (r w) -> p r w', w=W))
            else:
                nc.scalar.copy(cfg['dst_flat'][:, c * CH:(c + 1) * CH], pm[:, :])

    if cfg.get('dbg'):
        nc.sync.dma_start(cfg['dbg']['h1pre'].ap(), cfg['dst_plane'])

    # ---- BN stats over own rows ----
    stats_sum = sb.tile([128, 1], F32, tag='ssum')
    stats_sq = sb.tile([128, 1], F32, tag='ssq')
    if cfg['dst_plane'] is not None:
        pl3 = cfg['dst_plane'][:, :].rearrange('p (r w) -> p r w', w=WP)
        own = pl3[:, L2R0:L2R0 + OWN, MC:MC + W]
        scr = cfg['scratch'][:, 0:OWN * W].rearrange('p (r w) -> p r w', w=W)
        nc.scalar.activation(scr, own, ACTF.Copy, accum_out=stats_sum[:, :])
        nc.scalar.activation(scr, own, ACTF.Square, accum_out=stats_sq[:, :])
    else:
        src_f = cfg['dst_flat'][:, 0:px_all]
        scr = cfg['scratch'][:, 0:px_all]
        nc.scalar.activation(scr, src_f, ACTF.Copy, accum_out=stats_sum[:, :])
        nc.scalar.activation(scr, src_f, ACTF.Square, accum_out=stats_sq[:, :])

    # ---- AllReduce stats ----
    cc_in = dram.tile([128, 2], F32, tag=f'ccin{L}')
    cc_out = dram.tile([128, 2], F32, tag=f'ccout{L}')
    st2 = sb.tile([128, 2], F32, tag='st2')
    nc.vector.tensor_copy(st2[:, 0:1], stats_sum[:, :])
    nc.vector.tensor_copy(st2[:, 1:2], stats_sq[:, :])
    nc.gpsimd.dma_start(cc_in[:, :], st2[:, :])
    nc.gpsimd.collective_compute(
        "AllReduce", ALU.add, replica_groups=[list(range(NCORES))],
        ins=[cc_in[:, :].opt()], outs=[cc_out[:, :].opt()])
    nc.gpsimd.dma_start(st2[:, :], cc_out[:, :])

    # ---- scale/bias ----
    mean = sb.tile([128, 1], F32, tag='mean')
    var = sb.tile([128, 1], F32, tag='var')
    scl = sb.tile([128, 1], F32, tag=f'scl{L}')
    bia = sb.tile([128, 1], F32, tag=f'bia{L}')
    nc.vector.tensor_scalar(mean[:, :], st2[:, 0:1], 1.0 / CNT, None, ALU.mult)
    nc.vector.tensor_scalar(var[:, :], st2[:, 1:2], 1.0 / CNT, None, ALU.mult)
    nc.vector.tensor_tensor(scl[:, :], mean[:, :], mean[:, :], ALU.mult)
    nc.vector.tensor_tensor(var[:, :], var[:, :], scl[:, :], ALU.subtract)
    nc.vector.tensor_scalar(var[:, :], var[:, :], EPS, None, ALU.add)
    nc.scalar.sqrt(scl[:, :], var[:, :])
    nc.vector.reciprocal(scl[:, :], scl[:, :])
    nc.vector.tensor_tensor(scl[:, :], scl[:, :], cfg['gamma'][:, :], ALU.mult)
    nc.vector.tensor_tensor(bia[:, :], mean[:, :], scl[:, :], ALU.mult)
    nc.vector.tensor_tensor(bia[:, :], cfg['beta'][:, :], bia[:, :], ALU.subtract)

    # ---- BN apply + ReLU ----
    if cfg['dst_plane'] is not None:
        pl3 = cfg['dst_plane'][:, :].rearrange('p (r w) -> p r w', w=WP)
        own3 = pl3[:, L2R0:L2R0 + OWN, MC:MC + W]
        nc.scalar.activation(own3, own3, ACTF.Relu, scale=scl[:, :], bias=bia[:, :])
        # halo rows: BN then zero where out-of-image (topv/botv in {0,1})
        sclt = sb.tile([128, 1], F32, tag='sclt')
        biat = sb.tile([128, 1], F32, tag='biat')
        sclb = sb.tile([128, 1], F32, tag='sclb')
        biab = sb.tile([128, 1], F32, tag='biab')
        nc.vector.tensor_tensor(sclt[:, :], scl[:, :], cfg['topv'][:, :], ALU.mult)
        nc.vector.tensor_tensor(biat[:, :], bia[:, :], cfg['topv'][:, :], ALU.mult)
        nc.vector.tensor_tensor(sclb[:, :], scl[:, :], cfg['botv'][:, :], ALU.mult)
        nc.vector.tensor_tensor(biab[:, :], bia[:, :], cfg['botv'][:, :], ALU.mult)
        top3 = pl3[:, L1R0:L1R0 + 4, MC:MC + W]
        bot3 = pl3[:, L2R0 + OWN:L2R0 + OWN + 4, MC:MC + W]
        nc.scalar.activation(top3, top3, ACTF.Relu, scale=sclt[:, :], bias=biat[:, :])
        nc.scalar.activation(bot3, bot3, ACTF.Relu, scale=sclb[:, :], bias=biab[:, :])
    else:
        dst = cfg['dst_flat'][:, 0:px_all]
        nc.scalar.activation(dst, dst, ACTF.Relu, scale=scl[:, :], bias=bia[:, :])


def build_module(dbg=False):
    nc = bacc.Bacc(trn_type="TRN2", target_bir_lowering=False, debug=False,
                   num_devices=NCORES)

    d_in = {}
    for name, shape in [

            ('gy1', [K * L1NC, CH]), ('gx1', [K * L1NC, CH]), ('mb1', [K * L1NC, 1]),
            ('gy2', [K * L2NC, CH]), ('gx2', [K * L2NC, CH]), ('mb2', [K * L2NC, 1]),
            ('topv', [128, 1]), ('botv', [128, 1]),
            ('g1', [128, 1]), ('b1', [128, 1]), ('g2', [128, 1]), ('b2', [128, 1])]:
        d_in[name] = nc.dram_tensor(name, shape, F32, kind="ExternalInput")
    d_in['x_p'] = nc.dram_tensor('x_p', [CIN, PLANE], BF16, kind="ExternalInput")
    d_in['x_pf'] = nc.dram_tensor('x_pf', [2 * CIN, PLANE], F32, kind="ExternalInput")
    for nm, shp in [('offw1', [K, CIN, 27]), ('offw2', [K, CMID, 27]),
                    ('w1p', [5, 128, 128]), ('w2p', [K, 128, 128])]:
        d_in[nm] = nc.dram_tensor(nm, shp, BF16, kind="ExternalInput")
    d_in['vsel1'] = nc.dram_tensor('vsel1', [20, 36, 128], BF16, kind="ExternalInput")
    d_in['vsel2'] = nc.dram_tensor('vsel2', [36, 36, 128], BF16, kind="ExternalInput")
    d_out = nc.dram_tensor('out_c', [COUT, L2PX], F32, kind="ExternalOutput")
    d_dbg = {}
    if dbg:
        d_dbg['h1'] = nc.dram_tensor('dbg_h1', [CMID, PLANE], F32, kind="ExternalOutput")
        d_dbg['py'] = nc.dram_tensor('dbg_py', [K * L1NC, CH], F32, kind="ExternalOutput")
        d_dbg['px'] = nc.dram_tensor('dbg_px', [K * L1NC, CH], F32, kind="ExternalOutput")
        d_dbg['m'] = nc.dram_tensor('dbg_m', [K * L1NC, CH], F32, kind="ExternalOutput")
        d_dbg['idx'] = nc.dram_tensor('dbg_idx', [K * L1NC, CH], I16, kind="ExternalOutput")
        d_dbg['v36'] = nc.dram_tensor('dbg_v36', [36, L1PX], BF16, kind="ExternalOutput")
        d_dbg['wr'] = nc.dram_tensor('dbg_wr', [128, K * L1NG * 48], I16, kind="ExternalOutput")
        d_dbg['h1pre'] = nc.dram_tensor('dbg_h1pre', [CMID, PLANE], F32, kind="ExternalOutput")
        d_dbg['off'] = nc.dram_tensor('dbg_off', [27, L1PX], BF16, kind="ExternalOutput")
        d_dbg['G0'] = nc.dram_tensor('dbg_G0', [128, GRP], F32, kind="ExternalOutput")
        d_dbg['pv0'] = nc.dram_tensor('dbg_pv0', [128, GRP], F32, kind="ExternalOutput")
        d_dbg['S0'] = nc.dram_tensor('dbg_S0', [128, GRP], BF16, kind="ExternalOutput")
        d_dbg['dyst'] = nc.dram_tensor('dbg_dyst', [K * L1NC, CH], BF16, kind="ExternalOutput")
        d_dbg['dxst'] = nc.dram_tensor('dbg_dxst', [K * L1NC, CH], BF16, kind="ExternalOutput")

    with tile.TileContext(nc) as tc:
        with tc.tile_pool(name='sb', bufs=1) as sb_p, \
             tc.tile_pool(name='rot', bufs=3) as rot_p, \
             tc.tile_pool(name='psum', bufs=2, space="PSUM") as psum_p, \
             tc.tile_pool(name='dram', bufs=1, space="DRAM") as dram_p:

            pools = {'sb': sb_p, 'rot': rot_p, 'psum': psum_p, 'dram': dram_p}

            x_sb = sb_p.tile([CIN, PLANE], BF16, tag='x_sb')
            nc.sync.dma_start(x_sb[:, :], d_in['x_p'].ap())
            x_sf = sb_p.tile([2 * CIN, PLANE], F32, tag='x_sf')
            nc.sync.dma_start(x_sf[:, :], d_in['x_pf'].ap())
            h1_plane = sb_p.tile([CMID, PLANE], F32, tag='h1_plane')
            nc.vector.memset(h1_plane[:, :], 0.0)
            h1_bf = sb_p.tile([CMID, PLANE], BF16, tag='h1_bf')
            out2_sb = sb_p.tile([COUT, L2PX], F32, tag='out2_sb')

            def load(name, shape, dtype=F32):
                t = sb_p.tile(shape, dtype, tag=name)
                nc.sync.dma_start(t[0:shape[0], :], d_in[name].ap())
                return t

            gy1 = load('gy1', [K * L1NC, CH])
            gx1 = load('gx1', [K * L1NC, CH])
            mb1 = load('mb1', [K * L1NC, 1])
            gy2 = load('gy2', [K * L2NC, CH])
            gx2 = load('gx2', [K * L2NC, CH])
            mb2 = load('mb2', [K * L2NC, 1])
            ow1 = sb_p.tile([CIN, K * 27], BF16, tag='ow1')
            nc.sync.dma_start(ow1[:, :].rearrange('c (k o) -> c k o', o=27),
                              d_in['offw1'].ap().rearrange('k c o -> c k o'))
            ow2 = sb_p.tile([CMID, K * 27], BF16, tag='ow2')
            nc.sync.dma_start(ow2[:, :].rearrange('c (k o) -> c k o', o=27),
                              d_in['offw2'].ap().rearrange('k c o -> c k o'))
            w1p = sb_p.tile([128, 5 * 128], BF16, tag='w1p')
            nc.sync.dma_start(w1p[:, :].rearrange('r (b o) -> r b o', o=128),
                              d_in['w1p'].ap().rearrange('b r o -> r b o'))
            w2p = sb_p.tile([128, K * 128], BF16, tag='w2p')
            nc.sync.dma_start(w2p[:, :].rearrange('r (b o) -> r b o', o=128),
                              d_in['w2p'].ap().rearrange('b r o -> r b o'))
            vsel1 = sb_p.tile([36, 20 * 128], BF16, tag='vsel1')
            nc.sync.dma_start(vsel1[:, :].rearrange('r (b o) -> r b o', o=128),
                              d_in['vsel1'].ap().rearrange('b r o -> r b o'))
            vsel2 = sb_p.tile([36, 36 * 128], BF16, tag='vsel2')
            nc.sync.dma_start(vsel2[:, :].rearrange('r (b o) -> r b o', o=128),
                              d_in['vsel2'].ap().rearrange('b r o -> r b o'))
            topv = load('topv', [128, 1])
            botv = load('botv', [128, 1])
            g1 = load('g1', [128, 1])
            b1 = load('b1', [128, 1])
            g2 = load('g2', [128, 1])
            b2 = load('b2', [128, 1])

            blocks1 = [(w1p[:, b * 128:(b + 1) * 128], [2 * b, 2 * b + 1], 128)
                       for b in range(4)]
            blocks1.append((w1p[:, 4 * 128:5 * 128], [8], 64))
            blocks2 = [(w2p[:, k * 128:(k + 1) * 128], [k], 128) for k in range(K)]

            _deform_layer(nc, pools, dict(
                layer=1, cin=CIN, src=x_sb[:, :], gsrc=x_sf[:, :], offw=ow1[:, :], gy=gy1[:, :],
                gx=gx1[:, :], mb=mb1[:, :],
                wblocks=blocks1, nchunks=L1NC, ngroups=L1NG, prow0=L1R0,
                gamma=g1[:, :], beta=b1[:, :], topv=topv[:, :], botv=botv[:, :],
                dst_plane=h1_plane[:, :], dst_flat=None, scratch=out2_sb[:, :],
                vsel=vsel1[:, :], dbg=d_dbg if dbg else None))
            nc.vector.tensor_copy(h1_bf[:, :], h1_plane[:, :])
            if dbg:
                nc.sync.dma_start(d_dbg['h1'].ap(), h1_plane[:, :])

            _deform_layer(nc, pools, dict(
                layer=2, cin=CMID, src=h1_bf[:, :], gsrc=h1_plane[:, :], offw=ow2[:, :], gy=gy2[:, :],
                gx=gx2[:, :], mb=mb2[:, :],
                wblocks=blocks2, nchunks=L2NC, ngroups=L2NG, prow0=L2R0,
                gamma=g2[:, :], beta=b2[:, :], topv=topv[:, :], botv=botv[:, :],
                dst_plane=None, dst_flat=out2_sb[:, :], scratch=h1_plane[:, :],
                vsel=vsel2[:, :], dbg=None))

            nc.sync.dma_start(d_out.ap(), out2_sb[:, :])

    nc.compile()
    return nc


# ---------------- public entry ----------------
_CACHED = {}


def kernel(**inputs) -> np.ndarray:
    if 'nc' not in _CACHED:
        _CACHED['nc'] = build_module()
    nc = _CACHED['nc']
    in_maps = _host_prep(inputs)
    res = bass_utils.run_bass_kernel_spmd(nc, in_maps, core_ids=list(range(NCORES)))
    out = np.zeros((N, COUT, H, W), np.float32)
    for core in range(NCORES):
        n, half = core // 2, core % 2
        r0 = half * OWN
        out[n, :, r0:r0 + OWN, :] = res.results[core]['out_c'].reshape(COUT, OWN, W)
    return out



# revision 11
# speedup vs baseline: 3.1440x; 3.1440x over previous
"""Trainium2 Bass kernel for nn_DoubleConv (modulated deformable conv v2 x2 + BN + ReLU).

Sharding: data-parallel over (sample n, image half) -> 8 shards on 8 NeuronCores.
Each core computes both layers for its 48-row slice (with recomputed halo rows for
layer-2 sampling); training-mode BatchNorm statistics are made exact with a tiny
cross-core AllReduce of per-channel (sum, sumsq).

The bilinear sampling gather is the bottleneck (gpsimd ap_gather pays ~102 cycles
per 4 indices regardless of payload), so the source plane is stored quad-interleaved
(the 4 bilinear corners of every position contiguous) and gathered with d=4: one
index fetches all 4 corners, cutting the index count 4x vs a per-corner gather.

Self-contained: hardcodes all shapes from the problem spec.
"""

import numpy as np

import concourse.bass as bass
import concourse.bacc as bacc
import concourse.mybir as mybir
import concourse.tile as tile
from concourse import bass_utils

F32 = mybir.dt.float32
BF16 = mybir.dt.bfloat16
I16 = mybir.dt.int16
ALU = mybir.AluOpType
ACTF = mybir.ActivationFunctionType

# ---------------- geometry ----------------
N, CIN, CMID, COUT, H, W = 4, 64, 128, 128, 96, 96
K = 9
NCORES = 8
OWN = 48                      # own image rows per core
MR, MC = 8, 4                 # plane row/col margins
WP = W + 2 * MC               # 104 padded width
PH = OWN + 2 * MR             # 64 plane rows
PLANE = PH * WP               # 6656
L1R0, L1NR = 4, 56            # layer-1 computed plane rows [4, 60)
L2R0, L2NR = 8, 48            # layer-2 (own) plane rows [8, 56)
L1PX = L1NR * W               # 5376
L2PX = L2NR * W               # 4608
CH = 384                      # pixel chunk (4 rows x 96)
L1NC, L2NC = L1PX // CH, L2PX // CH   # 14, 12 chunks
GRP = 1152                    # gather group (3 chunks)
NE = PLANE - WP - 2           # max sampling index bound (quad stays in plane)
CNT = float(N * H * W)        # BN count 36864
EPS = 1e-5
QSH = [0, 1, WP, WP + 1]      # quad slot -> flat shift


def _plane_pad(img, r0):
    """img [C, 96, 96] -> padded plane [C, PH, WP] for own rows [r0, r0+48)."""
    C = img.shape[0]
    out = np.zeros((C, PH, WP), np.float32)
    lo, hi = r0 - MR, r0 + OWN + MR
    slo, shi = max(lo, 0), min(hi, H)
    out[:, slo - lo:shi - lo, MC:MC + W] = img[:, slo:shi, :]
    return out


def _host_prep(inputs):
    """Build the 8 per-core input maps (all numpy)."""
    import ml_dtypes as mld
    x = np.asarray(inputs['x'], np.float32)
    w1 = np.asarray(inputs['w1'], np.float32)
    off_w1 = np.asarray(inputs['off_w1'], np.float32)
    off_b1 = np.asarray(inputs['off_b1'], np.float32)
    g1 = np.asarray(inputs['gamma1'], np.float32)
    b1 = np.asarray(inputs['beta1'], np.float32)
    w2 = np.asarray(inputs['w2'], np.float32)
    off_w2 = np.asarray(inputs['off_w2'], np.float32)
    off_b2 = np.asarray(inputs['off_b2'], np.float32)
    g2 = np.asarray(inputs['gamma2'], np.float32)
    b2 = np.asarray(inputs['beta2'], np.float32)

    ky = np.arange(K) // 3 - 1
    kx = np.arange(K) % 3 - 1

    # offset conv weights, output channels permuted to (py x9, px x9, mlogit x9)
    perm = list(range(0, 18, 2)) + list(range(1, 18, 2)) + list(range(18, 27))

    def off_lhsT(ow, cin):
        owp = ow[perm]                       # [27, cin, 3, 3]
        t = np.zeros((K, cin, 27), np.float32)
        for t_i in range(K):
            ty, tx = t_i // 3 - 1, t_i % 3 - 1
            t[t_i] = owp[:, :, ty + 1, tx + 1].T
        return t.astype(mld.bfloat16)        # [9, cin, 27]

    offw1_t = off_lhsT(off_w1, CIN)
    offw2_t = off_lhsT(off_w2, CMID)

    # main conv lhsT blocks
    w1k = w1.reshape(CMID, CIN, K)
    w2k = w2.reshape(COUT, CMID, K)
    w1p = np.zeros((5, 128, 128), np.float32)
    for b in range(4):
        w1p[b, :64] = w1k[:, :, 2 * b].T
        w1p[b, 64:] = w1k[:, :, 2 * b + 1].T
    w1p[4, :64] = w1k[:, :, 8].T
    w1p = w1p.astype(mld.bfloat16)
    w2p = np.stack([w2k[:, :, k].T for k in range(K)]).astype(mld.bfloat16)

    # broadcast-matmul identity for V replication (one-hot column selects v36 row)
    id36 = np.eye(36, dtype=np.float32).astype(mld.bfloat16)

    # stacked per-pixel constant maps, layout [(k, chunk), CH]
    def grids(r0, nrows, prow0, nch, offb):
        pr = prow0 + np.arange(nrows)              # plane rows
        pc = MC + np.arange(W)                     # plane cols
        gy = np.broadcast_to(pr[:, None], (nrows, W)).reshape(-1).astype(np.float32)
        gx = np.broadcast_to(pc[None, :], (nrows, W)).reshape(-1).astype(np.float32)
        gy_st = np.zeros((K * nch, CH), np.float32)
        gx_st = np.zeros((K * nch, CH), np.float32)
        for k in range(K):
            for c in range(nch):
                gy_st[k * nch + c] = gy[c * CH:(c + 1) * CH] + ky[k] + offb[2 * k]
                gx_st[k * nch + c] = gx[c * CH:(c + 1) * CH] + kx[k] + offb[2 * k + 1]
        return gy_st, gx_st

    def quad_plane(flat64):
        """flat [64, PLANE] f32 -> quad-interleaved bf16 [128, PLANE*4] (replicated)."""
        ext = np.zeros((64, PLANE + WP + 2), np.float32)
        ext[:, :PLANE] = flat64
        q = np.empty((64, PLANE, 4), np.float32)
        for s, sh in enumerate(QSH):
            q[:, :, s] = ext[:, sh:sh + PLANE]
        q = q.reshape(64, PLANE * 4)
        return np.concatenate([q, q], 0).astype(mld.bfloat16)

    in_maps = []
    for core in range(NCORES):
        n, half = core // 2, core % 2
        r0 = half * OWN
        gy1, gx1 = grids(r0, L1NR, L1R0, L1NC, off_b1)
        gy2, gx2 = grids(r0, L2NR, L2R0, L2NC, off_b2)
        mb1 = np.repeat(off_b1[18:27], L1NC).astype(np.float32)[:, None]
        mb2 = np.repeat(off_b2[18:27], L2NC).astype(np.float32)[:, None]

        topv = np.full((128, 1), 0.0 if r0 == 0 else 1.0, np.float32)
        botv = np.full((128, 1), 0.0 if r0 + OWN >= H else 1.0, np.float32)

        in_maps.append({
            'x_q': quad_plane(_plane_pad(x[n], r0).reshape(CIN, PLANE)),
            'gy1': gy1, 'gx1': gx1, 'mb1': mb1,
            'gy2': gy2, 'gx2': gx2, 'mb2': mb2,
            'offw1': offw1_t, 'offw2': offw2_t,
            'w1p': w1p, 'w2p': w2p,
            'id36': id36,
            'topv': topv, 'botv': botv,
            'g1': g1[:, None].copy(), 'b1': b1[:, None].copy(),
            'g2': g2[:, None].copy(), 'b2': b2[:, None].copy(),
        })
    return in_maps


# ---------------- module build ----------------

def _q4(t):
    """[p, PLANE*4] tile AP -> 4D quad view [p, row, col, slot]."""
    return t.rearrange('p (r w q) -> p r w q', w=WP, q=4)


def _deform_layer(nc, pools, cfg):
    """Emit one modulated-deformable-conv layer + BN stats/apply."""
    cin = cfg['cin']
    nch = cfg['nchunks']
    nk_st = K * nch                    # stacked rows (126 / 108)
    px_all = nch * CH
    prow0 = cfg['prow0']
    wseg = px_all // 16                # wrapped idx cols per tap/block
    sb, g2p, s3p, psum, psum1, dram = (pools['sb'], pools['g2'], pools['s3'],
                                       pools['psum'], pools['psum1'], pools['dram'])
    L = cfg['layer']
    blocks = cfg['wblocks']
    nblk = len(blocks)

    # ---- offset conv: 9 accumulated matmuls per chunk -> DRAM (f32) ----
    dB = dram.tile([27, px_all], F32, tag=f'dB{L}')
    srcq = _q4(cfg['src'])
    for c in range(nch):
        po = psum.tile([27, CH], F32, tag='psum_off')
        for t in range(K):
            ty, tx = t // 3 - 1, t % 3 - 1
            rhs = srcq[0:cin, prow0 + 4 * c + ty: prow0 + 4 * c + ty + 4,
                       MC + tx: MC + tx + W, 0]
            lhsT = cfg['offw'][0:cin, t * 27:(t + 1) * 27]
            nc.tensor.matmul(po[:, :], lhsT, rhs,
                             start=(t == 0), stop=(t == K - 1))
        offst = s3p.tile([27, CH], F32, tag='offst')
        nc.scalar.copy(offst[:, :], po[:, :])
        nc.sync.dma_start(dB[:, c * CH:(c + 1) * CH], offst[:, :])

    # ---- stack (k,chunk) onto partitions ----
    dy_st = sb.tile([126, CH], F32, tag='dy_st')
    dx_st = sb.tile([126, CH], F32, tag='dx_st')
    ml_st = sb.tile([126, CH], F32, tag='ml_st')
    for (dst, p0) in ((dy_st, 0), (dx_st, 9), (ml_st, 18)):
        src = dB[p0:p0 + 9, :].rearrange('k (c u) -> (k c) u', c=nch)
        nc.sync.dma_start(dst[0:nk_st, :], src)

    # ---- per-pixel prep on stacked tiles ----
    # gy/gx are loaded per layer into shared tiles and consumed in place.
    gy = sb.tile([126, CH], F32, tag='gy')
    gx = sb.tile([126, CH], F32, tag='gx')
    nc.sync.dma_start(gy[0:nk_st, :], cfg['gy'].ap())
    nc.sync.dma_start(gx[0:nk_st, :], cfg['gx'].ap())
    y0 = sb.tile([126, CH], F32, tag='y0')
    x0 = sb.tile([126, CH], F32, tag='x0')
    m_st = sb.tile([126, CH], F32, tag='m_st')
    tmp = sb.tile([126, CH], F32, tag='tmp')
    wx0 = sb.tile([126, CH], F32, tag='wx0')
    idxi = sb.tile([126, CH], I16, tag='idxi')
    V = sb.tile([126, 4 * CH], BF16, tag='V')

    A = lambda t: t[0:nk_st, :]
    py, px = gy, gx                      # in-place: grids become sample coords
    nc.vector.tensor_tensor(A(py), A(dy_st), A(gy), ALU.add)
    nc.vector.tensor_tensor(A(px), A(dx_st), A(gx), ALU.add)
    # floor via round-to-nearest magic + compare (py, px always > 0 here)
    MAGIC = 12582912.0
    nc.vector.tensor_scalar(A(y0), A(py), MAGIC, None, ALU.add)
    nc.vector.tensor_scalar(A(y0), A(y0), -MAGIC, None, ALU.add)
    nc.vector.tensor_tensor(A(tmp), A(y0), A(py), ALU.is_gt)
    nc.vector.tensor_tensor(A(y0), A(y0), A(tmp), ALU.subtract)
    nc.vector.tensor_scalar(A(x0), A(px), MAGIC, None, ALU.add)
    nc.vector.tensor_scalar(A(x0), A(x0), -MAGIC, None, ALU.add)
    nc.vector.tensor_tensor(A(tmp), A(x0), A(px), ALU.is_gt)
    nc.vector.tensor_tensor(A(x0), A(x0), A(tmp), ALU.subtract)
    ly, lx = py, px                      # in-place: coords become lerp weights
    nc.vector.tensor_tensor(A(ly), A(py), A(y0), ALU.subtract)
    nc.vector.tensor_tensor(A(lx), A(px), A(x0), ALU.subtract)
    nc.scalar.activation(A(m_st), A(ml_st), ACTF.Sigmoid, bias=cfg['mb'][0:nk_st, :])
    # idx00 = y0*WP + x0, clamped to [0, NE-1]
    idxf = y0                            # in-place: y0 becomes flat index
    nc.vector.tensor_scalar(A(idxf), A(y0), float(WP), None, ALU.mult)
    nc.vector.tensor_tensor(A(idxf), A(idxf), A(x0), ALU.add)
    nc.vector.tensor_scalar(A(idxf), A(idxf), 0.0, float(NE - 1), ALU.max, ALU.min)
    nc.vector.tensor_copy(idxi[0:nk_st, :], A(idxf))

    # V[:, ab*CH:(ab+1)*CH] = m * wy_a * wx_b
    nc.vector.tensor_scalar(A(tmp), A(ly), 1.0, -1.0, ALU.subtract, ALU.mult)
    nc.vector.tensor_tensor(A(tmp), A(tmp), A(m_st), ALU.mult)    # m*(1-ly)
    mly = x0                             # in-place: x0 becomes m*ly
    nc.vector.tensor_tensor(A(mly), A(ly), A(m_st), ALU.mult)
    nc.vector.tensor_scalar(A(wx0), A(lx), 1.0, -1.0, ALU.subtract, ALU.mult)
    nc.vector.tensor_tensor(V[0:nk_st, 0 * CH:1 * CH], A(tmp), A(wx0), ALU.mult)
    nc.vector.tensor_tensor(V[0:nk_st, 1 * CH:2 * CH], A(tmp), A(lx), ALU.mult)
    nc.vector.tensor_tensor(V[0:nk_st, 2 * CH:3 * CH], A(mly), A(wx0), ALU.mult)
    nc.vector.tensor_tensor(V[0:nk_st, 3 * CH:4 * CH], A(mly), A(lx), ALU.mult)

    # ---- V36 [(ab,k), px_all] via DRAM reshape hop ----
    dV = dram.tile([126, 4 * CH], BF16, tag=f'dV{L}')
    nc.sync.dma_start(dV[0:nk_st, :], V[0:nk_st, :])
    v36 = sb.tile([36, 5376], BF16, tag='v36')
    for ab in range(4):
        src = dV[0:nk_st, ab * CH:(ab + 1) * CH].rearrange('(k c) u -> k c u', c=nch)
        dst = v36[ab * 9:ab * 9 + 9, 0:px_all].rearrange('k (c u) -> k c u', c=nch)
        nc.sync.dma_start(dst, src)

    # ---- wrapped int16 indices via DRAM hop ----
    dA = dram.tile([126, CH], I16, tag=f'dA{L}')
    nc.sync.dma_start(dA[0:nk_st, :], idxi[0:nk_st, :])
    wrapped = sb.tile([128, 2592], I16, tag='wrapped')
    dAr = dA[0:nk_st, :].rearrange('(k c) (u1 p) -> p k c u1', c=nch, u1=CH // 16)
    if cin == 64:
        # paired-tap layout: partitions 0-63 tap t_lo of block, 64-127 tap t_hi
        for b, (_, taps, rows) in enumerate(blocks):
            cs = slice(b * wseg, (b + 1) * wseg)
            d_lo = wrapped[0:16, cs].rearrange('p (c u1) -> p c u1', c=nch)
            d_hi = wrapped[64:80, cs].rearrange('p (c u1) -> p c u1', c=nch)
            nc.sync.dma_start(d_lo, dAr[:, taps[0]])
            nc.sync.dma_start(d_hi, dAr[:, taps[-1]])
        nw = nblk * wseg
        nc.sync.dma_start(wrapped[16:32, 0:nw], wrapped[0:16, 0:nw])
        nc.sync.dma_start(wrapped[32:64, 0:nw], wrapped[0:32, 0:nw])
        nc.sync.dma_start(wrapped[80:96, 0:nw], wrapped[64:80, 0:nw])
        nc.sync.dma_start(wrapped[96:128, 0:nw], wrapped[64:96, 0:nw])
    else:
        nw = K * wseg
        dst = wrapped[0:16, 0:nw].rearrange('p (k c u1) -> p k c u1', c=nch, u1=CH // 16)
        nc.sync.dma_start(dst, dAr)
        nc.sync.dma_start(wrapped[16:32, 0:nw], wrapped[0:16, 0:nw])
        nc.sync.dma_start(wrapped[32:64, 0:nw], wrapped[0:32, 0:nw])
        nc.sync.dma_start(wrapped[64:128, 0:nw], wrapped[0:64, 0:nw])

    # ---- per group: one d=4 gather per block; corner-weighted sum; matmuls ----
    groups = []
    p0 = 0
    while p0 < px_all:
        groups.append((p0, min(GRP, px_all - p0)))
        p0 += GRP
    for (gp0, gn) in groups:
        pms = []
        for b, (wl, taps, rows) in enumerate(blocks):
            Gq = g2p.tile([128, 4 * GRP], BF16, tag='Gq')
            nc.gpsimd.ap_gather(
                Gq[0:128, 0:4 * gn],
                cfg['gsrc'][0:128, :],
                wrapped[0:128, b * wseg + gp0 // 16: b * wseg + (gp0 + gn) // 16],
                channels=128, num_elems=PLANE, d=4, num_idxs=gn)
            Gq3 = Gq[:, :].rearrange('p (i q) -> p i q', q=4)
            S = s3p.tile([128, GRP], BF16, tag='S')
            for c in range(gn // CH):
                cs = slice(c * CH, (c + 1) * CH)
                vcols = slice(gp0 + c * CH, gp0 + (c + 1) * CH)
                for ab in range(4):
                    pv = psum.tile([128, CH], F32, tag='psum_vrep')
                    id36 = cfg['id36']
                    r_lo = ab * 9 + taps[0]
                    r_hi = ab * 9 + taps[-1]
                    if r_lo == r_hi:
                        lhsT = id36[0:36, r_lo:r_lo + 1].broadcast_to([36, 128])
                        nc.tensor.matmul(pv[:, :], lhsT, v36[0:36, vcols],
                                         start=True, stop=True)
                    else:
                        lhsT = id36[0:36, r_lo:r_lo + 1].broadcast_to([36, 64])
                        nc.tensor.matmul(pv[0:64, :], lhsT, v36[0:36, vcols],
                                         start=True, stop=True)
                        lhsT = id36[0:36, r_hi:r_hi + 1].broadcast_to([36, 64])
                        nc.tensor.matmul(pv[64:128, :], lhsT, v36[0:36, vcols],
                                         start=True, stop=True)
                    Gab = Gq3[0:rows, c * CH:(c + 1) * CH, ab]
                    if ab == 0:
                        nc.vector.tensor_tensor(S[0:rows, cs], Gab,
                                                pv[0:rows, :], ALU.mult)
                    else:
                        T2 = s3p.tile([128, CH], BF16, tag='Tbuf')
                        nc.vector.tensor_tensor(T2[0:rows, :], Gab,
                                                pv[0:rows, :], ALU.mult)
                        nc.vector.tensor_tensor(S[0:rows, cs], S[0:rows, cs],
                                                T2[0:rows, :], ALU.add)
            for c in range(gn // CH):
                if b == 0:
                    pm_c = psum1.tile([128, CH], F32, tag=f'psum_m{c}')
                    pms.append(pm_c)
                nc.tensor.matmul(pms[c][:, :], wl[0:rows, :],
                                 S[0:rows, c * CH:(c + 1) * CH],
                                 start=(b == 0), stop=(b == nblk - 1))
        for c in range(gn // CH):
            gc = (gp0 + c * CH) // CH       # global chunk
            if cfg['dst_q'] is not None:
                dst = _q4(cfg['dst_q'])[:, prow0 + 4 * gc: prow0 + 4 * gc + 4,
                                        MC:MC + W, 0]
                nc.scalar.copy(dst, pms[c][:, :].rearrange('p (r w) -> p r w', w=W))
            else:
                nc.scalar.copy(cfg['dst_flat'][:, gc * CH:(gc + 1) * CH], pms[c][:, :])

    # ---- BN stats over own rows ----
    stats_sum = sb.tile([128, 1], F32, tag='ssum')
    stats_sq = sb.tile([128, 1], F32, tag='ssq')
    if cfg['dst_q'] is not None:
        own = _q4(cfg['dst_q'])[:, L2R0:L2R0 + OWN, MC:MC + W, 0]
        scr = cfg['scratch'][:, 0:OWN * W].rearrange('p (r w) -> p r w', w=W)
        nc.scalar.activation(scr, own, ACTF.Copy, accum_out=stats_sum[:, :])
        nc.scalar.activation(scr, own, ACTF.Square, accum_out=stats_sq[:, :])
    else:
        src_f = cfg['dst_flat'][:, 0:px_all]
        scr = cfg['scratch'][:, 0:px_all]
        nc.scalar.activation(scr, src_f, ACTF.Copy, accum_out=stats_sum[:, :])
        nc.scalar.activation(scr, src_f, ACTF.Square, accum_out=stats_sq[:, :])

    # ---- AllReduce stats ----
    cc_in = dram.tile([128, 2], F32, tag=f'ccin{L}')
    cc_out = dram.tile([128, 2], F32, tag=f'ccout{L}')
    st2 = sb.tile([128, 2], F32, tag='st2')
    nc.vector.tensor_copy(st2[:, 0:1], stats_sum[:, :])
    nc.vector.tensor_copy(st2[:, 1:2], stats_sq[:, :])
    nc.gpsimd.dma_start(cc_in[:, :], st2[:, :])
    nc.gpsimd.collective_compute(
        "AllReduce", ALU.add, replica_groups=[list(range(NCORES))],
        ins=[cc_in[:, :].opt()], outs=[cc_out[:, :].opt()])
    nc.gpsimd.dma_start(st2[:, :], cc_out[:, :])

    # ---- scale/bias ----
    mean = sb.tile([128, 1], F32, tag='mean')
    var = sb.tile([128, 1], F32, tag='var')
    scl = sb.tile([128, 1], F32, tag=f'scl{L}')
    bia = sb.tile([128, 1], F32, tag=f'bia{L}')
    nc.vector.tensor_scalar(mean[:, :], st2[:, 0:1], 1.0 / CNT, None, ALU.mult)
    nc.vector.tensor_scalar(var[:, :], st2[:, 1:2], 1.0 / CNT, None, ALU.mult)
    nc.vector.tensor_tensor(scl[:, :], mean[:, :], mean[:, :], ALU.mult)
    nc.vector.tensor_tensor(var[:, :], var[:, :], scl[:, :], ALU.subtract)
    nc.vector.tensor_scalar(var[:, :], var[:, :], EPS, None, ALU.add)
    nc.scalar.sqrt(scl[:, :], var[:, :])
    nc.vector.reciprocal(scl[:, :], scl[:, :])
    nc.vector.tensor_tensor(scl[:, :], scl[:, :], cfg['gamma'][:, :], ALU.mult)
    nc.vector.tensor_tensor(bia[:, :], mean[:, :], scl[:, :], ALU.mult)
    nc.vector.tensor_tensor(bia[:, :], cfg['beta'][:, :], bia[:, :], ALU.subtract)

    # ---- BN apply + ReLU ----
    if cfg['dst_q'] is not None:
        plq = _q4(cfg['dst_q'])
        own3 = plq[:, L2R0:L2R0 + OWN, MC:MC + W, 0]
        nc.scalar.activation(own3, own3, ACTF.Relu, scale=scl[:, :], bias=bia[:, :])
        # halo rows: BN then zero where out-of-image (topv/botv in {0,1})
        sclt = sb.tile([128, 1], F32, tag='sclt')
        biat = sb.tile([128, 1], F32, tag='biat')
        sclb = sb.tile([128, 1], F32, tag='sclb')
        biab = sb.tile([128, 1], F32, tag='biab')
        nc.vector.tensor_tensor(sclt[:, :], scl[:, :], cfg['topv'][:, :], ALU.mult)
        nc.vector.tensor_tensor(biat[:, :], bia[:, :], cfg['topv'][:, :], ALU.mult)
        nc.vector.tensor_tensor(sclb[:, :], scl[:, :], cfg['botv'][:, :], ALU.mult)
        nc.vector.tensor_tensor(biab[:, :], bia[:, :], cfg['botv'][:, :], ALU.mult)
        top3 = plq[:, L1R0:L1R0 + 4, MC:MC + W, 0]
        bot3 = plq[:, L2R0 + OWN:L2R0 + OWN + 4, MC:MC + W, 0]
        nc.scalar.activation(top3, top3, ACTF.Relu, scale=sclt[:, :], bias=biat[:, :])
        nc.scalar.activation(bot3, bot3, ACTF.Relu, scale=sclb[:, :], bias=biab[:, :])
        # rebuild quad slots 1-3 from the BN'd slot 0 (shifted copies)
        fl = cfg['dst_q'].rearrange('p (i q) -> p i q', q=4)
        nlast = PLANE - WP - 1
        nc.scalar.copy(fl[:, 0:nlast, 1], fl[:, 1:nlast + 1, 0])
        nc.vector.tensor_copy(fl[:, 0:nlast, 2], fl[:, WP:nlast + WP, 0])
        nc.scalar.copy(fl[:, 0:nlast, 3], fl[:, WP + 1:nlast + WP + 1, 0])
    else:
        dst = cfg['dst_flat'][:, 0:px_all]
        nc.scalar.activation(dst, dst, ACTF.Relu, scale=scl[:, :], bias=bia[:, :])


def build_module():
    nc = bacc.Bacc(trn_type="TRN2", target_bir_lowering=False, debug=False,
                   num_devices=NCORES)

    d_in = {}
    for name, shape in [
            ('gy1', [K * L1NC, CH]), ('gx1', [K * L1NC, CH]), ('mb1', [K * L1NC, 1]),
            ('gy2', [K * L2NC, CH]), ('gx2', [K * L2NC, CH]), ('mb2', [K * L2NC, 1]),
            ('topv', [128, 1]), ('botv', [128, 1]),
            ('g1', [128, 1]), ('b1', [128, 1]), ('g2', [128, 1]), ('b2', [128, 1])]:
        d_in[name] = nc.dram_tensor(name, shape, F32, kind="ExternalInput")
    d_in['x_q'] = nc.dram_tensor('x_q', [128, PLANE * 4], BF16, kind="ExternalInput")
    for nm, shp in [('offw1', [K, CIN, 27]), ('offw2', [K, CMID, 27]),
                    ('w1p', [5, 128, 128]), ('w2p', [K, 128, 128]),
                    ('id36', [36, 36])]:
        d_in[nm] = nc.dram_tensor(nm, shp, BF16, kind="ExternalInput")
    d_out = nc.dram_tensor('out_c', [COUT, L2PX], F32, kind="ExternalOutput")

    with tile.TileContext(nc) as tc:
        with tc.tile_pool(name='sb', bufs=1) as sb_p, \
             tc.tile_pool(name='g2', bufs=2) as g2_p, \
             tc.tile_pool(name='s3', bufs=2) as s3_p, \
             tc.tile_pool(name='psum', bufs=2, space="PSUM") as psum_p, \
             tc.tile_pool(name='psum1', bufs=1, space="PSUM") as psum1_p, \
             tc.tile_pool(name='dram', bufs=1, space="DRAM") as dram_p:

            pools = {'sb': sb_p, 'g2': g2_p, 's3': s3_p,
                     'psum': psum_p, 'psum1': psum1_p, 'dram': dram_p}

            x_q = sb_p.tile([128, PLANE * 4], BF16, tag='x_q')
            nc.sync.dma_start(x_q[:, :], d_in['x_q'].ap())
            h1_q = sb_p.tile([128, PLANE * 4], BF16, tag='h1_q')
            nc.vector.memset(h1_q[:, :], 0.0)
            out2_sb = sb_p.tile([COUT, L2PX], F32, tag='out2_sb')

            def load(name, shape, dtype=F32):
                t = sb_p.tile(shape, dtype, tag=name)
                nc.sync.dma_start(t[0:shape[0], :], d_in[name].ap())
                return t

            mb1 = load('mb1', [K * L1NC, 1])
            mb2 = load('mb2', [K * L2NC, 1])
            ow1 = sb_p.tile([CIN, K * 27], BF16, tag='ow1')
            nc.sync.dma_start(ow1[:, :].rearrange('c (k o) -> c k o', o=27),
                              d_in['offw1'].ap().rearrange('k c o -> c k o'))
            ow2 = sb_p.tile([CMID, K * 27], BF16, tag='ow2')
            nc.sync.dma_start(ow2[:, :].rearrange('c (k o) -> c k o', o=27),
                              d_in['offw2'].ap().rearrange('k c o -> c k o'))
            w1p = sb_p.tile([128, 5 * 128], BF16, tag='w1p')
            nc.sync.dma_start(w1p[:, :].rearrange('r (b o) -> r b o', o=128),
                              d_in['w1p'].ap().rearrange('b r o -> r b o'))
            w2p = sb_p.tile([128, K * 128], BF16, tag='w2p')
            nc.sync.dma_start(w2p[:, :].rearrange('r (b o) -> r b o', o=128),
                              d_in['w2p'].ap().rearrange('b r o -> r b o'))
            id36 = sb_p.tile([36, 36], BF16, tag='id36')
            nc.sync.dma_start(id36[:, :], d_in['id36'].ap())
            topv = load('topv', [128, 1])
            botv = load('botv', [128, 1])
            g1 = load('g1', [128, 1])
            b1 = load('b1', [128, 1])
            g2 = load('g2', [128, 1])
            b2 = load('b2', [128, 1])

            blocks1 = [(w1p[:, b * 128:(b + 1) * 128], [2 * b, 2 * b + 1], 128)
                       for b in range(4)]
            blocks1.append((w1p[:, 4 * 128:5 * 128], [8], 64))
            blocks2 = [(w2p[:, k * 128:(k + 1) * 128], [k], 128) for k in range(K)]

            v36_t = None  # created inside layer; shared tag

            _deform_layer(nc, pools, dict(
                layer=1, cin=CIN, src=x_q[:, :], gsrc=x_q[:, :], offw=ow1[:, :],
                gy=d_in['gy1'], gx=d_in['gx1'], mb=mb1[:, :],
                wblocks=blocks1, nchunks=L1NC, prow0=L1R0,
                id36=id36[:, :],
                gamma=g1[:, :], beta=b1[:, :], topv=topv[:, :], botv=botv[:, :],
                dst_q=h1_q[:, :], dst_flat=None, scratch=out2_sb[:, :]))

            _deform_layer(nc, pools, dict(
                layer=2, cin=CMID, src=h1_q[:, :], gsrc=h1_q[:, :], offw=ow2[:, :],
                gy=d_in['gy2'], gx=d_in['gx2'], mb=mb2[:, :],
                wblocks=blocks2, nchunks=L2NC, prow0=L2R0,
                id36=id36[:, :],
                gamma=g2[:, :], beta=b2[:, :], topv=topv[:, :], botv=botv[:, :],
                dst_q=None, dst_flat=out2_sb[:, :], scratch=h1_q[:, :]))

            nc.sync.dma_start(d_out.ap(), out2_sb[:, :])

    nc.compile()
    return nc


# ---------------- public entry ----------------
_CACHED = {}


def kernel(**inputs) -> np.ndarray:
    if 'nc' not in _CACHED:
        _CACHED['nc'] = build_module()
    nc = _CACHED['nc']
    in_maps = _host_prep(inputs)
    res = bass_utils.run_bass_kernel_spmd(nc, in_maps, core_ids=list(range(NCORES)))
    out = np.zeros((N, COUT, H, W), np.float32)
    for core in range(NCORES):
        n, half = core // 2, core % 2
        r0 = half * OWN
        out[n, :, r0:r0 + OWN, :] = res.results[core]['out_c'].reshape(COUT, OWN, W)
    return out


# revision 15
# speedup vs baseline: 3.5654x; 1.1340x over previous
"""Trainium2 Bass kernel for nn_DoubleConv (modulated deformable conv v2 x2 + BN + ReLU).

Sharding: data-parallel over (sample n, image half) -> 8 shards on 8 NeuronCores.
Each core computes both layers for its 48-row slice (with recomputed halo rows for
layer-2 sampling); training-mode BatchNorm statistics are made exact with a tiny
cross-core AllReduce of per-channel (sum, sumsq).

The bilinear sampling gather is the bottleneck (gpsimd ap_gather pays ~102 cycles
per 4 indices regardless of payload), so the source plane is stored quad-interleaved
(the 4 bilinear corners of every position contiguous) and gathered with d=4: one
index fetches all 4 corners, cutting the index count 4x vs a per-corner gather.

Self-contained: hardcodes all shapes from the problem spec.
"""

import numpy as np

import concourse.bass as bass
import concourse.bacc as bacc
import concourse.mybir as mybir
import concourse.tile as tile
from concourse import bass_utils

F32 = mybir.dt.float32
BF16 = mybir.dt.bfloat16
I16 = mybir.dt.int16
ALU = mybir.AluOpType
ACTF = mybir.ActivationFunctionType

# ---------------- geometry ----------------
N, CIN, CMID, COUT, H, W = 4, 64, 128, 128, 96, 96
K = 9
NCORES = 8
OWN = 48                      # own image rows per core
MR, MC = 8, 4                 # plane row/col margins
WP = W + 2 * MC               # 104 padded width
PH = OWN + 2 * MR             # 64 plane rows
PLANE = PH * WP               # 6656
L1R0, L1NR = 4, 56            # layer-1 computed plane rows [4, 60)
L2R0, L2NR = 8, 48            # layer-2 (own) plane rows [8, 56)
L1PX = L1NR * W               # 5376
L2PX = L2NR * W               # 4608
CH = 384                      # pixel chunk (4 rows x 96)
L1NC, L2NC = L1PX // CH, L2PX // CH   # 14, 12 chunks
GRP = 1152                    # gather group (3 chunks)
NE = PLANE - WP - 2           # max sampling index bound (quad stays in plane)
CNT = float(N * H * W)        # BN count 36864
EPS = 1e-5
QSH = [0, 1, WP, WP + 1]      # quad slot -> flat shift


def _plane_pad(img, r0):
    """img [C, 96, 96] -> padded plane [C, PH, WP] for own rows [r0, r0+48)."""
    C = img.shape[0]
    out = np.zeros((C, PH, WP), np.float32)
    lo, hi = r0 - MR, r0 + OWN + MR
    slo, shi = max(lo, 0), min(hi, H)
    out[:, slo - lo:shi - lo, MC:MC + W] = img[:, slo:shi, :]
    return out


def _host_prep(inputs):
    """Build the 8 per-core input maps (all numpy)."""
    import ml_dtypes as mld
    x = np.asarray(inputs['x'], np.float32)
    w1 = np.asarray(inputs['w1'], np.float32)
    off_w1 = np.asarray(inputs['off_w1'], np.float32)
    off_b1 = np.asarray(inputs['off_b1'], np.float32)
    g1 = np.asarray(inputs['gamma1'], np.float32)
    b1 = np.asarray(inputs['beta1'], np.float32)
    w2 = np.asarray(inputs['w2'], np.float32)
    off_w2 = np.asarray(inputs['off_w2'], np.float32)
    off_b2 = np.asarray(inputs['off_b2'], np.float32)
    g2 = np.asarray(inputs['gamma2'], np.float32)
    b2 = np.asarray(inputs['beta2'], np.float32)

    ky = np.arange(K) // 3 - 1
    kx = np.arange(K) % 3 - 1

    # offset conv weights, output channels permuted to (py x9, px x9, mlogit x9)
    perm = list(range(0, 18, 2)) + list(range(1, 18, 2)) + list(range(18, 27))

    def off_lhsT(ow, cin):
        owp = ow[perm]                       # [27, cin, 3, 3]
        t = np.zeros((K, cin, 27), np.float32)
        for t_i in range(K):
            ty, tx = t_i // 3 - 1, t_i % 3 - 1
            t[t_i] = owp[:, :, ty + 1, tx + 1].T
        return t.astype(mld.bfloat16)        # [9, cin, 27]

    offw1_t = off_lhsT(off_w1, CIN)
    offw2_t = off_lhsT(off_w2, CMID)

    # main conv lhsT blocks
    w1k = w1.reshape(CMID, CIN, K)
    w2k = w2.reshape(COUT, CMID, K)
    w1p = np.zeros((5, 128, 128), np.float32)
    for b in range(4):
        w1p[b, :64] = w1k[:, :, 2 * b].T
        w1p[b, 64:] = w1k[:, :, 2 * b + 1].T
    w1p[4, :64] = w1k[:, :, 8].T
    w1p[4, 64:] = w1k[:, :, 8].T
    w1p = w1p.astype(mld.bfloat16)
    w2p = np.stack([w2k[:, :, k].T for k in range(K)]).astype(mld.bfloat16)

    # broadcast-matmul identity for V replication (one-hot column selects v36 row)
    id36 = np.eye(36, dtype=np.float32).astype(mld.bfloat16)

    # stacked per-pixel constant maps, layout [(k, chunk), CH]
    def grids(r0, nrows, prow0, nch, offb):
        pr = prow0 + np.arange(nrows)              # plane rows
        pc = MC + np.arange(W)                     # plane cols
        gy = np.broadcast_to(pr[:, None], (nrows, W)).reshape(-1).astype(np.float32)
        gx = np.broadcast_to(pc[None, :], (nrows, W)).reshape(-1).astype(np.float32)
        gy_st = np.zeros((K * nch, CH), np.float32)
        gx_st = np.zeros((K * nch, CH), np.float32)
        for k in range(K):
            for c in range(nch):
                gy_st[k * nch + c] = gy[c * CH:(c + 1) * CH] + ky[k] + offb[2 * k]
                gx_st[k * nch + c] = gx[c * CH:(c + 1) * CH] + kx[k] + offb[2 * k + 1]
        return gy_st, gx_st

    def quad_plane(flat64):
        """flat [64, PLANE] f32 -> quad-interleaved bf16 [128, PLANE*4] (replicated)."""
        ext = np.zeros((64, PLANE + WP + 2), np.float32)
        ext[:, :PLANE] = flat64
        q = np.empty((64, PLANE, 4), np.float32)
        for s, sh in enumerate(QSH):
            q[:, :, s] = ext[:, sh:sh + PLANE]
        q = q.reshape(64, PLANE * 4)
        return np.concatenate([q, q], 0).astype(mld.bfloat16)

    in_maps = []
    for core in range(NCORES):
        n, half = core // 2, core % 2
        r0 = half * OWN
        gy1, gx1 = grids(r0, L1NR, L1R0, L1NC, off_b1)
        gy2, gx2 = grids(r0, L2NR, L2R0, L2NC, off_b2)
        mb1 = np.repeat(off_b1[18:27], L1NC).astype(np.float32)[:, None]
        mb2 = np.repeat(off_b2[18:27], L2NC).astype(np.float32)[:, None]

        topv = np.full((128, 1), 0.0 if r0 == 0 else 1.0, np.float32)
        botv = np.full((128, 1), 0.0 if r0 + OWN >= H else 1.0, np.float32)

        in_maps.append({
            'x_q': quad_plane(_plane_pad(x[n], r0).reshape(CIN, PLANE)),
            'gy1': gy1, 'gx1': gx1, 'mb1': mb1,
            'gy2': gy2, 'gx2': gx2, 'mb2': mb2,
            'offw1': offw1_t, 'offw2': offw2_t,
            'w1p': w1p, 'w2p': w2p,
            'id36': id36,
            'topv': topv, 'botv': botv,
            'g1': g1[:, None].copy(), 'b1': b1[:, None].copy(),
            'g2': g2[:, None].copy(), 'b2': b2[:, None].copy(),
        })
    return in_maps


# ---------------- module build ----------------

def _q4(t):
    """[p, PLANE*4] tile AP -> 4D quad view [p, row, col, slot]."""
    return t.rearrange('p (r w q) -> p r w q', w=WP, q=4)


def _deform_layer(nc, pools, cfg):
    """Emit one modulated-deformable-conv layer + BN stats/apply."""
    cin = cfg['cin']
    nch = cfg['nchunks']
    nk_st = K * nch                    # stacked rows (126 / 108)
    px_all = nch * CH
    prow0 = cfg['prow0']
    wseg = px_all // 16                # wrapped idx cols per tap/block
    sb, g2p, s3p, psum, psum1, dram = (pools['sb'], pools['g2'], pools['s3'],
                                       pools['psum'], pools['psum1'], pools['dram'])
    L = cfg['layer']
    blocks = cfg['wblocks']
    nblk = len(blocks)

    # ---- offset conv: 9 accumulated matmuls per chunk -> DRAM (f32) ----
    dB = dram.tile([27, px_all], F32, tag=f'dB{L}')
    srcq = _q4(cfg['src'])
    for c in range(nch):
        po = psum.tile([27, CH], F32, tag='psum_off')
        for t in range(K):
            ty, tx = t // 3 - 1, t % 3 - 1
            rhs = srcq[0:cin, prow0 + 4 * c + ty: prow0 + 4 * c + ty + 4,
                       MC + tx: MC + tx + W, 0]
            lhsT = cfg['offw'][0:cin, t * 27:(t + 1) * 27]
            nc.tensor.matmul(po[:, :], lhsT, rhs,
                             start=(t == 0), stop=(t == K - 1))
        offst = s3p.tile([27, CH], F32, tag='offst')
        nc.scalar.copy(offst[:, :], po[:, :])
        nc.sync.dma_start(dB[:, c * CH:(c + 1) * CH], offst[:, :])

    # ---- stack (k,chunk) onto partitions ----
    dy_st = sb.tile([126, CH], F32, tag='dy_st')
    dx_st = sb.tile([126, CH], F32, tag='dx_st')
    ml_st = sb.tile([126, CH], F32, tag='ml_st')
    for (dst, p0) in ((dy_st, 0), (dx_st, 9), (ml_st, 18)):
        src = dB[p0:p0 + 9, :].rearrange('k (c u) -> (k c) u', c=nch)
        nc.sync.dma_start(dst[0:nk_st, :], src)

    # ---- per-pixel prep on stacked tiles ----
    # gy/gx are loaded per layer into shared tiles and consumed in place.
    gy = sb.tile([126, CH], F32, tag='gy')
    gx = sb.tile([126, CH], F32, tag='gx')
    nc.sync.dma_start(gy[0:nk_st, :], cfg['gy'].ap())
    nc.sync.dma_start(gx[0:nk_st, :], cfg['gx'].ap())
    y0 = sb.tile([126, CH], F32, tag='y0')
    x0 = sb.tile([126, CH], F32, tag='x0')
    m_st = sb.tile([126, CH], F32, tag='m_st')
    tmp = sb.tile([126, CH], F32, tag='tmp')
    wx0 = sb.tile([126, CH], F32, tag='wx0')
    idxi = sb.tile([126, CH], I16, tag='idxi')
    V = sb.tile([126, 4 * CH], BF16, tag='V')

    A = lambda t: t[0:nk_st, :]
    py, px = gy, gx                      # in-place: grids become sample coords
    nc.vector.tensor_tensor(A(py), A(dy_st), A(gy), ALU.add)
    nc.vector.tensor_tensor(A(px), A(dx_st), A(gx), ALU.add)
    # floor via round-to-nearest magic + compare (py, px always > 0 here)
    MAGIC = 12582912.0
    nc.vector.tensor_scalar(A(y0), A(py), MAGIC, None, ALU.add)
    nc.vector.tensor_scalar(A(y0), A(y0), -MAGIC, None, ALU.add)
    nc.vector.tensor_tensor(A(tmp), A(y0), A(py), ALU.is_gt)
    nc.vector.tensor_tensor(A(y0), A(y0), A(tmp), ALU.subtract)
    nc.vector.tensor_scalar(A(x0), A(px), MAGIC, None, ALU.add)
    nc.vector.tensor_scalar(A(x0), A(x0), -MAGIC, None, ALU.add)
    nc.vector.tensor_tensor(A(tmp), A(x0), A(px), ALU.is_gt)
    nc.vector.tensor_tensor(A(x0), A(x0), A(tmp), ALU.subtract)
    ly, lx = py, px                      # in-place: coords become lerp weights
    nc.vector.tensor_tensor(A(ly), A(py), A(y0), ALU.subtract)
    nc.vector.tensor_tensor(A(lx), A(px), A(x0), ALU.subtract)
    nc.scalar.activation(A(m_st), A(ml_st), ACTF.Sigmoid, bias=cfg['mb'][0:nk_st, :])
    # idx00 = y0*WP + x0, clamped to [0, NE-1]
    idxf = y0                            # in-place: y0 becomes flat index
    nc.vector.tensor_scalar(A(idxf), A(y0), float(WP), None, ALU.mult)
    nc.vector.tensor_tensor(A(idxf), A(idxf), A(x0), ALU.add)
    nc.vector.tensor_scalar(A(idxf), A(idxf), 0.0, float(NE - 1), ALU.max, ALU.min)
    # store idxi wrap-transposed: idxi[:, p*24+u1] = idxf[:, u1*16+p] so the
    # DRAM->wrapped DMA below moves contiguous 24-element runs per descriptor
    dsti = idxi[0:nk_st, :].rearrange('r (p u) -> r p u', p=16)
    srci = A(idxf).rearrange('r (u p) -> r p u', u=CH // 16)
    nc.vector.tensor_copy(dsti, srci)

    # V[:, ab*CH:(ab+1)*CH] = m * wy_a * wx_b
    nc.vector.tensor_scalar(A(tmp), A(ly), 1.0, -1.0, ALU.subtract, ALU.mult)
    nc.vector.tensor_tensor(A(tmp), A(tmp), A(m_st), ALU.mult)    # m*(1-ly)
    mly = x0                             # in-place: x0 becomes m*ly
    nc.vector.tensor_tensor(A(mly), A(ly), A(m_st), ALU.mult)
    nc.vector.tensor_scalar(A(wx0), A(lx), 1.0, -1.0, ALU.subtract, ALU.mult)
    nc.vector.tensor_tensor(V[0:nk_st, 0 * CH:1 * CH], A(tmp), A(wx0), ALU.mult)
    nc.vector.tensor_tensor(V[0:nk_st, 1 * CH:2 * CH], A(tmp), A(lx), ALU.mult)
    nc.vector.tensor_tensor(V[0:nk_st, 2 * CH:3 * CH], A(mly), A(wx0), ALU.mult)
    nc.vector.tensor_tensor(V[0:nk_st, 3 * CH:4 * CH], A(mly), A(lx), ALU.mult)

    # ---- V36 [(ab,k), px_all] via DRAM reshape hop ----
    dV = dram.tile([126, 4 * CH], BF16, tag=f'dV{L}')
    nc.sync.dma_start(dV[0:nk_st, :], V[0:nk_st, :])
    v36 = sb.tile([36, 5376], BF16, tag='v36')
    for ab in range(4):
        src = dV[0:nk_st, ab * CH:(ab + 1) * CH].rearrange('(k c) u -> k c u', c=nch)
        dst = v36[ab * 9:ab * 9 + 9, 0:px_all].rearrange('k (c u) -> k c u', c=nch)
        nc.sync.dma_start(dst, src)

    # ---- wrapped int16 indices via DRAM hop ----
    dA = dram.tile([126, CH], I16, tag=f'dA{L}')
    nc.sync.dma_start(dA[0:nk_st, :], idxi[0:nk_st, :])
    wrapped = sb.tile([128, 2592], I16, tag='wrapped')
    dAr = dA[0:nk_st, :].rearrange('(k c) (p u1) -> p k c u1', c=nch, p=16)
    if cin == 64:
        # paired-tap layout: partitions 0-63 tap t_lo of block, 64-127 tap t_hi
        for b, (_, taps, rows) in enumerate(blocks):
            cs = slice(b * wseg, (b + 1) * wseg)
            d_lo = wrapped[0:16, cs].rearrange('p (c u1) -> p c u1', c=nch)
            nc.sync.dma_start(d_lo, dAr[:, taps[0]])
            if rows == 64:
                # split4: odd chunks of tap 8 at even-chunk column positions
                nco = nch - 1
                d_hi = wrapped[64:80, b * wseg:b * wseg + nco * 24].rearrange(
                    'p (c u1) -> p c u1', c=nco)
                nc.sync.dma_start(d_hi, dAr[:, taps[-1], 1:nch])
                # init the never-gathered tail column block too
                d_hp = wrapped[64:80, b * wseg + nco * 24:(b + 1) * wseg].rearrange(
                    'p (c u1) -> p c u1', c=1)
                nc.sync.dma_start(d_hp, dAr[:, taps[-1], nch - 1:nch])
            else:
                d_hi = wrapped[64:80, cs].rearrange('p (c u1) -> p c u1', c=nch)
                nc.sync.dma_start(d_hi, dAr[:, taps[-1]])
        nw = nblk * wseg
        nc.sync.dma_start(wrapped[16:32, 0:nw], wrapped[0:16, 0:nw])
        nc.sync.dma_start(wrapped[32:64, 0:nw], wrapped[0:32, 0:nw])
        nc.sync.dma_start(wrapped[80:96, 0:nw], wrapped[64:80, 0:nw])
        nc.sync.dma_start(wrapped[96:128, 0:nw], wrapped[64:96, 0:nw])
    else:
        nw = K * wseg
        dst = wrapped[0:16, 0:nw].rearrange('p (k c u1) -> p k c u1', c=nch, u1=CH // 16)
        nc.sync.dma_start(dst, dAr)
        nc.sync.dma_start(wrapped[16:32, 0:nw], wrapped[0:16, 0:nw])
        nc.sync.dma_start(wrapped[32:64, 0:nw], wrapped[0:32, 0:nw])
        nc.sync.dma_start(wrapped[64:128, 0:nw], wrapped[0:64, 0:nw])

    # ---- per group: one d=4 gather per block; corner-weighted sum; matmuls ----
    id36 = cfg['id36']
    grp = cfg['grp']

    def emit_pv(pv, prt, row, vcols):
        lhsT = id36[0:36, row:row + 1].broadcast_to([36, prt.stop - prt.start])
        nc.tensor.matmul(pv[prt, :], lhsT, v36[0:36, vcols], start=True, stop=True)

    groups = []
    p0 = 0
    while p0 < px_all:
        groups.append((p0, min(grp, px_all - p0)))
        p0 += grp
    for (gp0, gn) in groups:
        pms = []
        for b, (wl, taps, rows) in enumerate(blocks):
            split4 = (cin == 64 and rows == 64)
            n_idx = gn // 2 if split4 else gn
            Gq = g2p.tile([128, 4 * 1152], BF16, tag='Gq')
            nc.gpsimd.ap_gather(
                Gq[0:128, 0:4 * n_idx],
                cfg['gsrc'][0:128, :],
                wrapped[0:128, b * wseg + gp0 // 16:
                        b * wseg + gp0 // 16 + n_idx // 16],
                channels=128, num_elems=PLANE, d=4, num_idxs=n_idx)
            Gq3 = Gq[:, :].rearrange('p (i q) -> p i q', q=4)
            S = s3p.tile([128, 1152], BF16, tag='S')
            for c in range(gn // CH):
                vcols = slice(gp0 + c * CH, gp0 + (c + 1) * CH)
                if split4:
                    # tap 8 on both halves: partitions 0-63 hold even chunk,
                    # 64-127 the odd chunk (shifted index table)
                    prt = slice(0, 64) if c == 0 else slice(64, 128)
                    gcols = slice(0, CH)
                    scols = slice(0, CH)
                else:
                    prt = slice(0, 128)
                    gcols = slice(c * CH, (c + 1) * CH)
                    scols = gcols
                for ab in range(4):
                    pv = psum.tile([128, CH], F32, tag='psum_vrep')
                    r_lo = ab * 9 + taps[0]
                    r_hi = ab * 9 + taps[-1]
                    if split4 or r_lo == r_hi:
                        emit_pv(pv, prt, r_lo, vcols)
                    else:
                        emit_pv(pv, slice(0, 64), r_lo, vcols)
                        emit_pv(pv, slice(64, 128), r_hi, vcols)
                    Gab = Gq3[prt, gcols, ab]
                    if ab == 0:
                        nc.vector.tensor_tensor(S[prt, scols], Gab,
                                                pv[prt, :], ALU.mult)
                    else:
                        T2 = s3p.tile([128, CH], BF16, tag='Tbuf')
                        nc.vector.tensor_tensor(T2[prt, :], Gab,
                                                pv[prt, :], ALU.mult)
                        nc.vector.tensor_tensor(S[prt, scols], S[prt, scols],
                                                T2[prt, :], ALU.add)
            for c in range(gn // CH):
                if b == 0:
                    pm_c = psum1.tile([128, CH], F32, tag=f'psum_m{c}')
                    pms.append(pm_c)
                if split4:
                    prt = slice(0, 64) if c == 0 else slice(64, 128)
                    nc.tensor.matmul(pms[c][:, :], wl[prt, :], S[prt, 0:CH],
                                     start=(b == 0), stop=(b == nblk - 1))
                else:
                    nc.tensor.matmul(pms[c][:, :], wl[0:rows, :],
                                     S[0:rows, c * CH:(c + 1) * CH],
                                     start=(b == 0), stop=(b == nblk - 1))
        for c in range(gn // CH):
            gc = (gp0 + c * CH) // CH       # global chunk
            if cfg['dst_q'] is not None:
                dst = _q4(cfg['dst_q'])[:, prow0 + 4 * gc: prow0 + 4 * gc + 4,
                                        MC:MC + W, 0]
                nc.scalar.copy(dst, pms[c][:, :].rearrange('p (r w) -> p r w', w=W))
            else:
                nc.scalar.copy(cfg['dst_flat'][:, gc * CH:(gc + 1) * CH], pms[c][:, :])

    # ---- BN stats over own rows ----
    stats_sum = sb.tile([128, 1], F32, tag='ssum')
    stats_sq = sb.tile([128, 1], F32, tag='ssq')
    if cfg['dst_q'] is not None:
        own = _q4(cfg['dst_q'])[:, L2R0:L2R0 + OWN, MC:MC + W, 0]
        scr = cfg['scratch'][:, 0:OWN * W].rearrange('p (r w) -> p r w', w=W)
        nc.scalar.activation(scr, own, ACTF.Copy, accum_out=stats_sum[:, :])
        nc.scalar.activation(scr, own, ACTF.Square, accum_out=stats_sq[:, :])
    else:
        src_f = cfg['dst_flat'][:, 0:px_all]
        scr = cfg['scratch'][:, 0:px_all]
        nc.scalar.activation(scr, src_f, ACTF.Copy, accum_out=stats_sum[:, :])
        nc.scalar.activation(scr, src_f, ACTF.Square, accum_out=stats_sq[:, :])

    # ---- AllReduce stats ----
    cc_in = dram.tile([128, 2], F32, tag=f'ccin{L}')
    cc_out = dram.tile([128, 2], F32, tag=f'ccout{L}')
    st2 = sb.tile([128, 2], F32, tag='st2')
    nc.vector.tensor_copy(st2[:, 0:1], stats_sum[:, :])
    nc.vector.tensor_copy(st2[:, 1:2], stats_sq[:, :])
    nc.gpsimd.dma_start(cc_in[:, :], st2[:, :])
    nc.gpsimd.collective_compute(
        "AllReduce", ALU.add, replica_groups=[list(range(NCORES))],
        ins=[cc_in[:, :].opt()], outs=[cc_out[:, :].opt()])
    nc.gpsimd.dma_start(st2[:, :], cc_out[:, :])

    # ---- scale/bias ----
    mean = sb.tile([128, 1], F32, tag='mean')
    var = sb.tile([128, 1], F32, tag='var')
    scl = sb.tile([128, 1], F32, tag=f'scl{L}')
    bia = sb.tile([128, 1], F32, tag=f'bia{L}')
    nc.vector.tensor_scalar(mean[:, :], st2[:, 0:1], 1.0 / CNT, None, ALU.mult)
    nc.vector.tensor_scalar(var[:, :], st2[:, 1:2], 1.0 / CNT, None, ALU.mult)
    nc.vector.tensor_tensor(scl[:, :], mean[:, :], mean[:, :], ALU.mult)
    nc.vector.tensor_tensor(var[:, :], var[:, :], scl[:, :], ALU.subtract)
    nc.vector.tensor_scalar(var[:, :], var[:, :], EPS, None, ALU.add)
    nc.scalar.sqrt(scl[:, :], var[:, :])
    nc.vector.reciprocal(scl[:, :], scl[:, :])
    nc.vector.tensor_tensor(scl[:, :], scl[:, :], cfg['gamma'][:, :], ALU.mult)
    nc.vector.tensor_tensor(bia[:, :], mean[:, :], scl[:, :], ALU.mult)
    nc.vector.tensor_tensor(bia[:, :], cfg['beta'][:, :], bia[:, :], ALU.subtract)

    # ---- BN apply + ReLU ----
    if cfg['dst_q'] is not None:
        plq = _q4(cfg['dst_q'])
        own3 = plq[:, L2R0:L2R0 + OWN, MC:MC + W, 0]
        nc.scalar.activation(own3, own3, ACTF.Relu, scale=scl[:, :], bias=bia[:, :])
        # halo rows: BN then zero where out-of-image (topv/botv in {0,1})
        sclt = sb.tile([128, 1], F32, tag='sclt')
        biat = sb.tile([128, 1], F32, tag='biat')
        sclb = sb.tile([128, 1], F32, tag='sclb')
        biab = sb.tile([128, 1], F32, tag='biab')
        nc.vector.tensor_tensor(sclt[:, :], scl[:, :], cfg['topv'][:, :], ALU.mult)
        nc.vector.tensor_tensor(biat[:, :], bia[:, :], cfg['topv'][:, :], ALU.mult)
        nc.vector.tensor_tensor(sclb[:, :], scl[:, :], cfg['botv'][:, :], ALU.mult)
        nc.vector.tensor_tensor(biab[:, :], bia[:, :], cfg['botv'][:, :], ALU.mult)
        top3 = plq[:, L1R0:L1R0 + 4, MC:MC + W, 0]
        bot3 = plq[:, L2R0 + OWN:L2R0 + OWN + 4, MC:MC + W, 0]
        nc.scalar.activation(top3, top3, ACTF.Relu, scale=sclt[:, :], bias=biat[:, :])
        nc.scalar.activation(bot3, bot3, ACTF.Relu, scale=sclb[:, :], bias=biab[:, :])
        # rebuild quad slots 1-3 from the BN'd slot 0 (shifted copies)
        fl = cfg['dst_q'].rearrange('p (i q) -> p i q', q=4)
        nlast = PLANE - WP - 1
        nc.scalar.copy(fl[:, 0:nlast, 1], fl[:, 1:nlast + 1, 0])
        nc.vector.tensor_copy(fl[:, 0:nlast, 2], fl[:, WP:nlast + WP, 0])
        nc.scalar.copy(fl[:, 0:nlast, 3], fl[:, WP + 1:nlast + WP + 1, 0])
    else:
        dst = cfg['dst_flat'][:, 0:px_all]
        nc.scalar.activation(dst, dst, ACTF.Relu, scale=scl[:, :], bias=bia[:, :])


def build_module():
    nc = bacc.Bacc(trn_type="TRN2", target_bir_lowering=False, debug=False,
                   num_devices=NCORES)

    d_in = {}
    for name, shape in [
            ('gy1', [K * L1NC, CH]), ('gx1', [K * L1NC, CH]), ('mb1', [K * L1NC, 1]),
            ('gy2', [K * L2NC, CH]), ('gx2', [K * L2NC, CH]), ('mb2', [K * L2NC, 1]),
            ('topv', [128, 1]), ('botv', [128, 1]),
            ('g1', [128, 1]), ('b1', [128, 1]), ('g2', [128, 1]), ('b2', [128, 1])]:
        d_in[name] = nc.dram_tensor(name, shape, F32, kind="ExternalInput")
    d_in['x_q'] = nc.dram_tensor('x_q', [128, PLANE * 4], BF16, kind="ExternalInput")
    for nm, shp in [('offw1', [K, CIN, 27]), ('offw2', [K, CMID, 27]),
                    ('w1p', [5, 128, 128]), ('w2p', [K, 128, 128]),
                    ('id36', [36, 36])]:
        d_in[nm] = nc.dram_tensor(nm, shp, BF16, kind="ExternalInput")
    d_out = nc.dram_tensor('out_c', [COUT, L2PX], F32, kind="ExternalOutput")

    with tile.TileContext(nc) as tc:
        with tc.tile_pool(name='sb', bufs=1) as sb_p, \
             tc.tile_pool(name='g2', bufs=2) as g2_p, \
             tc.tile_pool(name='s3', bufs=2) as s3_p, \
             tc.tile_pool(name='psum', bufs=2, space="PSUM") as psum_p, \
             tc.tile_pool(name='psum1', bufs=1, space="PSUM") as psum1_p, \
             tc.tile_pool(name='dram', bufs=1, space="DRAM") as dram_p:

            pools = {'sb': sb_p, 'g2': g2_p, 's3': s3_p,
                     'psum': psum_p, 'psum1': psum1_p, 'dram': dram_p}

            x_q = sb_p.tile([128, PLANE * 4], BF16, tag='x_q')
            nc.sync.dma_start(x_q[:, 0:PLANE * 2], d_in['x_q'].ap()[:, 0:PLANE * 2])
            nc.sync.dma_start(x_q[:, PLANE * 2:], d_in['x_q'].ap()[:, PLANE * 2:])
            h1_q = sb_p.tile([128, PLANE * 4], BF16, tag='h1_q')
            nc.vector.memset(h1_q[:, :], 0.0)
            out2_sb = sb_p.tile([COUT, L2PX], F32, tag='out2_sb')

            def load(name, shape, dtype=F32):
                t = sb_p.tile(shape, dtype, tag=name)
                nc.sync.dma_start(t[0:shape[0], :], d_in[name].ap())
                return t

            mb1 = load('mb1', [K * L1NC, 1])
            mb2 = load('mb2', [K * L2NC, 1])
            ow1 = sb_p.tile([CIN, K * 27], BF16, tag='ow1')
            nc.sync.dma_start(ow1[:, :].rearrange('c (k o) -> c k o', o=27),
                              d_in['offw1'].ap().rearrange('k c o -> c k o'))
            ow2 = sb_p.tile([CMID, K * 27], BF16, tag='ow2')
            nc.sync.dma_start(ow2[:, :].rearrange('c (k o) -> c k o', o=27),
                              d_in['offw2'].ap().rearrange('k c o -> c k o'))
            w1p = sb_p.tile([128, 5 * 128], BF16, tag='w1p')
            nc.sync.dma_start(w1p[:, :].rearrange('r (b o) -> r b o', o=128),
                              d_in['w1p'].ap().rearrange('b r o -> r b o'))
            w2p = sb_p.tile([128, K * 128], BF16, tag='w2p')
            nc.sync.dma_start(w2p[:, :].rearrange('r (b o) -> r b o', o=128),
                              d_in['w2p'].ap().rearrange('b r o -> r b o'))
            id36 = sb_p.tile([36, 36], BF16, tag='id36')
            nc.sync.dma_start(id36[:, :], d_in['id36'].ap())
            topv = load('topv', [128, 1])
            botv = load('botv', [128, 1])
            g1 = load('g1', [128, 1])
            b1 = load('b1', [128, 1])
            g2 = load('g2', [128, 1])
            b2 = load('b2', [128, 1])

            blocks1 = [(w1p[:, b * 128:(b + 1) * 128], [2 * b, 2 * b + 1], 128)
                       for b in range(4)]
            blocks1.append((w1p[:, 4 * 128:5 * 128], [8], 64))
            blocks2 = [(w2p[:, k * 128:(k + 1) * 128], [k], 128) for k in range(K)]

            v36_t = None  # created inside layer; shared tag

            _deform_layer(nc, pools, dict(
                layer=1, cin=CIN, src=x_q[:, :], gsrc=x_q[:, :], offw=ow1[:, :],
                gy=d_in['gy1'], gx=d_in['gx1'], mb=mb1[:, :],
                wblocks=blocks1, nchunks=L1NC, prow0=L1R0, grp=768,
                id36=id36[:, :],
                gamma=g1[:, :], beta=b1[:, :], topv=topv[:, :], botv=botv[:, :],
                dst_q=h1_q[:, :], dst_flat=None, scratch=out2_sb[:, :]))

            _deform_layer(nc, pools, dict(
                layer=2, cin=CMID, src=h1_q[:, :], gsrc=h1_q[:, :], offw=ow2[:, :],
                gy=d_in['gy2'], gx=d_in['gx2'], mb=mb2[:, :],
                wblocks=blocks2, nchunks=L2NC, prow0=L2R0, grp=1152,
                id36=id36[:, :],
                gamma=g2[:, :], beta=b2[:, :], topv=topv[:, :], botv=botv[:, :],
                dst_q=None, dst_flat=out2_sb[:, :], scratch=h1_q[:, :]))

            nc.sync.dma_start(d_out.ap(), out2_sb[:, :])

    nc.compile()
    return nc


# ---------------- public entry ----------------
_CACHED = {}


def kernel(**inputs) -> np.ndarray:
    if 'nc' not in _CACHED:
        _CACHED['nc'] = build_module()
    nc = _CACHED['nc']
    in_maps = _host_prep(inputs)
    res = bass_utils.run_bass_kernel_spmd(nc, in_maps, core_ids=list(range(NCORES)))
    out = np.zeros((N, COUT, H, W), np.float32)
    for core in range(NCORES):
        n, half = core // 2, core % 2
        r0 = half * OWN
        out[n, :, r0:r0 + OWN, :] = res.results[core]['out_c'].reshape(COUT, OWN, W)
    return out
